# revision 24
# baseline (speedup 1.0000x reference)
import sys

for _p in ('/opt/trn_rl_repo', '/root/.axon_site'):
    if _p not in sys.path:
        sys.path.insert(0, _p)

import numpy as np

B, H, W = 8, 512, 512
K = 3
NCORES = 8
# padded image: 1 zero row/col before, 2 zero rows/cols after (cols padded
# further so shifted views stay in range and rows stay 4B-aligned)
HP, WP = H + 3, W + 8
NBLK = 4          # row blocks of 128 partitions packed along the free dim
AW = 520          # A tile width (Ipad cols 0..519)
DW = 516          # Dx/Dy/Dxy tile width

# GpSimd tensor_tensor measured: mutual serialization with DVE TT on the
# shared SBUF port pair (combined throughput BELOW DVE alone) — keep all
# elementwise on DVE. GpSimd only generates SWDGE descriptors for the
# CCE-accumulate DMAs below.
CCE_ADD_TAPS = (2, 3, 4, 5, 6)   # taps whose t2 += Dy runs on the DMA CCE adder
DEFER = 3                        # taps to defer a CCE tap's final mul by

_compiled = None


def _build():
    import concourse.bacc as bacc
    import concourse.mybir as mybir
    from concourse.tile import TileContext

    f32, f16 = mybir.dt.float32, mybir.dt.float16
    ALU = mybir.AluOpType
    ACTF = mybir.ActivationFunctionType

    nc = bacc.Bacc("TRN2", target_bir_lowering=False, debug=False,
                   num_devices=NCORES)
    ipad = nc.dram_tensor("ipad", [HP, WP], f16, kind="ExternalInput")
    # offsets host-cast to fp16 and pre-packed to the SBUF tile layout:
    # offp[p, k, e, j, c] = offset[2k+e, 128j+p, c]  (e: 0=ly, 1=lx)
    offp = nc.dram_tensor("offp", [128, K * K, 2, NBLK, W], f16,
                          kind="ExternalInput")
    # stack of diag(w_k) matrices used as PE stationary weights
    wdg = nc.dram_tensor("wdg", [128, K * K, 128], f16, kind="ExternalInput")
    out = nc.dram_tensor("out", [H, W], f32, kind="ExternalOutput")

    with TileContext(nc) as tc:
        with (
            tc.tile_pool(name="img", bufs=1) as ip,
            tc.tile_pool(name="lylx", bufs=6) as lp,
            tc.tile_pool(name="tmp", bufs=DEFER + 2) as tp,
            tc.tile_pool(name="cst", bufs=1) as cp,
            tc.tile_pool(name="psum", bufs=1, space="PSUM") as pp,
        ):
            # image row-shifted copies (the two HWDGE rings)
            A = {}

            # per-tap (ly, lx) pair: one plain HWDGE load each, fp16 in HBM
            lylx = {}

            def load_lylx(k, eng):
                lylx[k] = lp.tile([128, 2, NBLK, W], f16, tag="l",
                                  name=f"l{k}")
                eng.dma_start(out=lylx[k][:], in_=offp[:, k])

            # j-half image chunks interleaved with offset pairs, in the
            # order the DVE stream consumes them — no engine waits long
            def load_img_half(dy, h, eng):
                if dy not in A:
                    A[dy] = ip.tile([128, NBLK, AW], f16, tag=f"A{dy}",
                                    name=f"A{dy}")
                eng.dma_start(
                    out=A[dy][:, 2 * h:2 * h + 2],
                    in_=ipad[dy + 1:dy + 513, 0:AW].rearrange(
                        "(j p) c -> p j c", p=128)[:, 2 * h:2 * h + 2])

            load_img_half(-1, 0, nc.sync)
            load_img_half(-1, 1, nc.scalar)
            load_img_half(0, 0, nc.sync)
            load_img_half(0, 1, nc.scalar)
            load_lylx(0, nc.sync)
            load_img_half(1, 0, nc.scalar)
            load_img_half(1, 1, nc.sync)
            load_lylx(1, nc.scalar)
            load_img_half(2, 0, nc.sync)
            load_img_half(2, 1, nc.scalar)
            load_lylx(2, nc.sync)
            wd = cp.tile([128, K * K, 128], f16, name="wd")
            nc.scalar.dma_start(out=wd[:], in_=wdg[:])
            for k in range(3, K * K):
                load_lylx(k, nc.sync if k % 2 == 0 else nc.scalar)

            # one PSUM tile per row block so bank j's drain never
            # serializes against bank j+1's matmuls
            psum = [pp.tile([128, W], f32, tag=f"ps{j}", name=f"ps{j}")
                    for j in range(NBLK)]

            Dx, Dy, Dxy = {}, {}, {}
            for dy in (-1, 0, 1, 2):
                Dx[dy] = ip.tile([128, NBLK, DW], f16, tag=f"D{dy}",
                                 name=f"D{dy}")
            for j in (-1, 0, 1):
                Dy[j] = ip.tile([128, NBLK, DW], f16, tag=f"Y{j}",
                                name=f"Y{j}")
                Dxy[j] = ip.tile([128, NBLK, DW], f16, tag=f"X{j}",
                                 name=f"X{j}")

            def hs(h):
                return slice(2 * h, 2 * h + 2)

            def make_dx(dy, h):
                # Dx = horizontal difference of the padded image
                nc.vector.tensor_tensor(Dx[dy][:, hs(h)],
                                        A[dy][:, hs(h), 1:1 + DW],
                                        A[dy][:, hs(h), 0:DW], ALU.subtract)

            def make_dy(j, h):
                # Dy = vertical difference of the padded image
                nc.vector.tensor_tensor(Dy[j][:, hs(h)],
                                        A[j + 1][:, hs(h), 0:DW],
                                        A[j][:, hs(h), 0:DW], ALU.subtract)

            def make_dxy(j, h):
                # Dxy = vertical difference of Dx (cross term)
                nc.vector.tensor_tensor(Dxy[j][:, hs(h)],
                                        Dx[j + 1][:, hs(h)],
                                        Dx[j][:, hs(h)], ALU.subtract)

            def iview(dy, q):
                return A[dy][:, :, q:q + W]

            outv = out.rearrange("(j p) c -> p j c", p=128)

            def emit_mms(k, ky, q, t, t2, j):
                wk = wd[:, k, :]
                last = k == K * K - 1
                nc.tensor.matmul(psum[j][:], wk, iview(ky, q)[:, j, :],
                                 start=(k == 0), stop=False)
                nc.tensor.matmul(psum[j][:], wk, t[:, j, :],
                                 start=False, stop=False)
                nc.tensor.matmul(psum[j][:], wk, t2[:, j, :],
                                 start=False, stop=last)
                if last:
                    # bank j is final: drain it while later banks finish
                    res = cp.tile([128, W], f32, tag=f"res{j}",
                                  name=f"res{j}")
                    nc.scalar.activation(res[:], psum[j][:], ACTF.Copy)
                    eng = nc.sync if j % 2 == 0 else nc.scalar
                    eng.dma_start(out=outv[:, j], in_=res[:])

            # per-row prep, j-half granularity: row ky's differences are
            # emitted just before its taps, so rows 0/1 prep hides inside
            # the tap stream while the next offset pairs arrive
            prepped = set()

            def prep_row(ky):
                for h in range(2):
                    if (ky, 'x') not in prepped:
                        make_dx(ky, h)
                    if (ky + 1, 'x') not in prepped:
                        make_dx(ky + 1, h)
                    make_dy(ky, h)
                    make_dxy(ky, h)
                prepped.add((ky, 'x'))
                prepped.add((ky + 1, 'x'))

            # per tap: v*w_k = w_k*I0 + w_k*m0 + w_k*u
            #   m0 = lx*Dx[ky]
            #   u  = ly*(Dy[ky] + lx*Dxy[ky])
            # CCE taps run t2 += Dy on the DMA adder; their final mul (and
            # matmuls) are deferred DEFER taps so the strict-FIFO DVE queue
            # has covering work while the DMA add completes.
            pending = []

            def flush_pending(limit):
                while pending and len(pending) > limit:
                    pk, pky, pq, pt, pt2, ply = pending.pop(0)
                    nc.vector.tensor_tensor(pt2[:], ply[:], pt2[:], ALU.mult)
                    for j in range(NBLK):
                        emit_mms(pk, pky, pq, pt, pt2, j)

            for k in range(K * K):
                ky, kx = k // K - 1, k % K - 1
                q = kx + 1
                if kx == -1:
                    prep_row(ky)
                ll = lylx[k]
                ly, lx = ll[:, 0], ll[:, 1]
                last = k == K * K - 1

                if last:
                    flush_pending(0)
                    # per-block ops so bank j's accumulation closes (and
                    # drains) while later blocks are still on the DVE
                    t = tp.tile([128, NBLK, W], f16, tag="t", name="t")
                    t2 = tp.tile([128, NBLK, W], f16, tag="t2", name="t2")
                    for j in range(NBLK):
                        nc.vector.tensor_tensor(
                            t[:, j], lx[:, j], Dx[ky][:, j, q:q + W],
                            ALU.mult)
                        nc.vector.tensor_tensor(
                            t2[:, j], lx[:, j], Dxy[ky][:, j, q:q + W],
                            ALU.mult)
                        nc.vector.tensor_tensor(
                            t2[:, j], t2[:, j], Dy[ky][:, j, q:q + W],
                            ALU.add)
                        nc.vector.tensor_tensor(t2[:, j], ly[:, j], t2[:, j],
                                                ALU.mult)
                        emit_mms(k, ky, q, t, t2, j)
                    continue

                t = tp.tile([128, NBLK, W], f16, tag="t", name="t")
                t2 = tp.tile([128, NBLK, W], f16, tag="t2", name="t2")
                nc.vector.tensor_tensor(t[:], lx[:], Dx[ky][:, :, q:q + W],
                                        ALU.mult)
                nc.vector.tensor_tensor(t2[:], lx[:], Dxy[ky][:, :, q:q + W],
                                        ALU.mult)
                if k in CCE_ADD_TAPS:
                    # t2 += Dy on the DMA engines' inline adder (SWDGE)
                    nc.gpsimd.dma_start(out=t2[:], in_=Dy[ky][:, :, q:q + W],
                                        accum_op=ALU.add)
                    pending.append((k, ky, q, t, t2, ly))
                    flush_pending(DEFER)
                else:
                    flush_pending(DEFER)
                    nc.vector.tensor_tensor(t2[:], t2[:],
                                            Dy[ky][:, :, q:q + W], ALU.add)
                    nc.vector.tensor_tensor(t2[:], ly[:], t2[:], ALU.mult)
                    for j in range(NBLK):
                        emit_mms(k, ky, q, t, t2, j)

    nc.compile()
    return nc


def kernel(input, weight, offset):
    global _compiled
    from concourse.bass_utils import run_bass_kernel_spmd

    if _compiled is None:
        _compiled = _build()
    nc = _compiled

    input = np.asarray(input, dtype=np.float32)
    offset = np.asarray(offset, dtype=np.float32)
    w9 = np.asarray(weight, dtype=np.float32).reshape(K * K)
    wdg = np.zeros((128, K * K, 128), np.float16)
    idx = np.arange(128)
    for k in range(K * K):
        wdg[idx, k, idx] = w9[k].astype(np.float16)

    ipad = np.zeros((B, HP, WP), np.float16)
    ipad[:, 1:H + 1, 1:W + 1] = input.astype(np.float16)

    # [B, 18, 512, 512] -> [B, p, k, e, j, c] fp16, contiguous per partition
    offp = np.ascontiguousarray(
        offset.reshape(B, K * K, 2, NBLK, 128, W).transpose(0, 4, 1, 2, 3, 5)
    ).astype(np.float16)

    in_maps = [
        {"ipad": ipad[b], "offp": offp[b], "wdg": wdg} for b in range(B)
    ]
    res = run_bass_kernel_spmd(nc, in_maps, list(range(NCORES)), trace=False)
    return np.stack([res.results[b]["out"] for b in range(B)], axis=0)


# revision 25
# speedup vs baseline: 1.0427x; 1.0427x over previous
import sys

for _p in ('/opt/trn_rl_repo', '/root/.axon_site'):
    if _p not in sys.path:
        sys.path.insert(0, _p)

import numpy as np

B, H, W = 8, 512, 512
K = 3
NCORES = 8
# padded image: 1 zero row/col before, 2 zero rows/cols after (cols padded
# further so shifted views stay in range and rows stay 4B-aligned)
HP, WP = H + 3, W + 8
NBLK = 4          # row blocks of 128 partitions packed along the free dim
AW = 520          # A tile width (Ipad cols 0..519)
DW = 516          # Dx/Dy/Dxy tile width

# GpSimd tensor_tensor measured: mutual serialization with DVE TT on the
# shared SBUF port pair (combined throughput BELOW DVE alone) — keep all
# elementwise on DVE. GpSimd only generates SWDGE descriptors for the
# CCE-accumulate DMAs below.
CCE_ADD_TAPS = (2, 3, 4, 5)   # taps whose t2 += Dy runs on the DMA CCE adder
DEFER = 3                        # taps to defer a CCE tap's final mul by

_compiled = None


def _build():
    import concourse.bacc as bacc
    import concourse.mybir as mybir
    from concourse.tile import TileContext

    f32, f16 = mybir.dt.float32, mybir.dt.float16
    ALU = mybir.AluOpType
    ACTF = mybir.ActivationFunctionType

    nc = bacc.Bacc("TRN2", target_bir_lowering=False, debug=False,
                   num_devices=NCORES)
    ipad = nc.dram_tensor("ipad", [HP, WP], f16, kind="ExternalInput")
    # offsets host-cast to fp16 and pre-packed to the SBUF tile layout:
    # offp[p, k, e, j, c] = offset[2k+e, 128j+p, c]  (e: 0=ly, 1=lx)
    offp = nc.dram_tensor("offp", [128, K * K, 2, NBLK, W], f16,
                          kind="ExternalInput")
    # stack of diag(w_k) matrices used as PE stationary weights
    wdg = nc.dram_tensor("wdg", [128, K * K, 128], f16, kind="ExternalInput")
    out = nc.dram_tensor("out", [H, W], f32, kind="ExternalOutput")

    with TileContext(nc) as tc:
        with (
            tc.tile_pool(name="img", bufs=1) as ip,
            tc.tile_pool(name="lylx", bufs=6) as lp,
            tc.tile_pool(name="tmp", bufs=DEFER + 2) as tp,
            tc.tile_pool(name="cst", bufs=1) as cp,
            tc.tile_pool(name="psum", bufs=1, space="PSUM") as pp,
        ):
            # image row-shifted copies (the two HWDGE rings)
            A = {}

            # per-tap (ly, lx) pair: one plain HWDGE load each, fp16 in HBM
            lylx = {}

            def load_lylx(k, eng):
                lylx[k] = lp.tile([128, 2, NBLK, W], f16, tag="l",
                                  name=f"l{k}")
                eng.dma_start(out=lylx[k][:], in_=offp[:, k])

            # j-half image chunks interleaved with offset pairs, in the
            # order the DVE stream consumes them — no engine waits long
            def load_img_half(dy, h, eng):
                if dy not in A:
                    A[dy] = ip.tile([128, NBLK, AW], f16, tag=f"A{dy}",
                                    name=f"A{dy}")
                eng.dma_start(
                    out=A[dy][:, 2 * h:2 * h + 2],
                    in_=ipad[dy + 1:dy + 513, 0:AW].rearrange(
                        "(j p) c -> p j c", p=128)[:, 2 * h:2 * h + 2])

            load_img_half(-1, 0, nc.sync)
            load_img_half(-1, 1, nc.scalar)
            load_img_half(0, 0, nc.sync)
            load_img_half(0, 1, nc.scalar)
            load_lylx(0, nc.sync)
            load_img_half(1, 0, nc.scalar)
            load_img_half(1, 1, nc.sync)
            load_lylx(1, nc.scalar)
            load_img_half(2, 0, nc.sync)
            load_img_half(2, 1, nc.scalar)
            load_lylx(2, nc.sync)
            wd = cp.tile([128, K * K, 128], f16, name="wd")
            nc.scalar.dma_start(out=wd[:], in_=wdg[:])
            for k in range(3, K * K):
                load_lylx(k, nc.sync if k % 2 == 0 else nc.scalar)

            # one PSUM tile per row block so bank j's drain never
            # serializes against bank j+1's matmuls
            psum = [pp.tile([128, W], f32, tag=f"ps{j}", name=f"ps{j}")
                    for j in range(NBLK)]

            Dx, Dy, Dxy = {}, {}, {}
            for dy in (-1, 0, 1, 2):
                Dx[dy] = ip.tile([128, NBLK, DW], f16, tag=f"D{dy}",
                                 name=f"D{dy}")
            for j in (-1, 0, 1):
                Dy[j] = ip.tile([128, NBLK, DW], f16, tag=f"Y{j}",
                                name=f"Y{j}")
                Dxy[j] = ip.tile([128, NBLK, DW], f16, tag=f"X{j}",
                                 name=f"X{j}")

            def hs(h):
                return slice(2 * h, 2 * h + 2)

            def make_dx(dy, h):
                # Dx = horizontal difference of the padded image
                nc.vector.tensor_tensor(Dx[dy][:, hs(h)],
                                        A[dy][:, hs(h), 1:1 + DW],
                                        A[dy][:, hs(h), 0:DW], ALU.subtract)

            def make_dy(j, h):
                # Dy = vertical difference of the padded image
                nc.vector.tensor_tensor(Dy[j][:, hs(h)],
                                        A[j + 1][:, hs(h), 0:DW],
                                        A[j][:, hs(h), 0:DW], ALU.subtract)

            def make_dxy(j, h):
                # Dxy = vertical difference of Dx (cross term)
                nc.vector.tensor_tensor(Dxy[j][:, hs(h)],
                                        Dx[j + 1][:, hs(h)],
                                        Dx[j][:, hs(h)], ALU.subtract)

            def iview(dy, q):
                return A[dy][:, :, q:q + W]

            outv = out.rearrange("(j p) c -> p j c", p=128)

            def emit_mms(k, ky, q, t, t2, j):
                wk = wd[:, k, :]
                last = k == K * K - 1
                nc.tensor.matmul(psum[j][:], wk, iview(ky, q)[:, j, :],
                                 start=(k == 0), stop=False)
                nc.tensor.matmul(psum[j][:], wk, t[:, j, :],
                                 start=False, stop=False)
                nc.tensor.matmul(psum[j][:], wk, t2[:, j, :],
                                 start=False, stop=last)
                if last:
                    # bank j is final: drain it while later banks finish
                    res = cp.tile([128, W], f32, tag=f"res{j}",
                                  name=f"res{j}")
                    nc.scalar.activation(res[:], psum[j][:], ACTF.Copy)
                    eng = nc.sync if j % 2 == 0 else nc.scalar
                    eng.dma_start(out=outv[:, j], in_=res[:])

            # per-row prep, j-half granularity: row ky's differences are
            # emitted just before its taps, so rows 0/1 prep hides inside
            # the tap stream while the next offset pairs arrive
            prepped = set()

            def prep_row(ky):
                for h in range(2):
                    if (ky, 'x') not in prepped:
                        make_dx(ky, h)
                    if (ky + 1, 'x') not in prepped:
                        make_dx(ky + 1, h)
                for h in range(2):
                    make_dy(ky, h)
                for h in range(2):
                    make_dxy(ky, h)
                prepped.add((ky, 'x'))
                prepped.add((ky + 1, 'x'))

            # per tap: v*w_k = w_k*I0 + w_k*m0 + w_k*u
            #   m0 = lx*Dx[ky]
            #   u  = ly*(Dy[ky] + lx*Dxy[ky])
            # CCE taps run t2 += Dy on the DMA adder; their final mul (and
            # matmuls) are deferred DEFER taps so the strict-FIFO DVE queue
            # has covering work while the DMA add completes.
            pending = []

            def flush_pending(limit):
                while pending and len(pending) > limit:
                    pk, pky, pq, pt, pt2, ply = pending.pop(0)
                    nc.vector.tensor_tensor(pt2[:], ply[:], pt2[:], ALU.mult)
                    for j in range(NBLK):
                        emit_mms(pk, pky, pq, pt, pt2, j)

            for k in range(K * K):
                ky, kx = k // K - 1, k % K - 1
                q = kx + 1
                if kx == -1:
                    prep_row(ky)
                ll = lylx[k]
                ly, lx = ll[:, 0], ll[:, 1]
                last = k == K * K - 1

                if last:
                    flush_pending(0)
                    # per-block ops so bank j's accumulation closes (and
                    # drains) while later blocks are still on the DVE
                    t = tp.tile([128, NBLK, W], f16, tag="t", name="t")
                    t2 = tp.tile([128, NBLK, W], f16, tag="t2", name="t2")
                    for j in range(NBLK):
                        nc.vector.tensor_tensor(
                            t[:, j], lx[:, j], Dx[ky][:, j, q:q + W],
                            ALU.mult)
                        nc.vector.tensor_tensor(
                            t2[:, j], lx[:, j], Dxy[ky][:, j, q:q + W],
                            ALU.mult)
                        nc.vector.tensor_tensor(
                            t2[:, j], t2[:, j], Dy[ky][:, j, q:q + W],
                            ALU.add)
                        nc.vector.tensor_tensor(t2[:, j], ly[:, j], t2[:, j],
                                                ALU.mult)
                        emit_mms(k, ky, q, t, t2, j)
                    continue

                t = tp.tile([128, NBLK, W], f16, tag="t", name="t")
                t2 = tp.tile([128, NBLK, W], f16, tag="t2", name="t2")
                nc.vector.tensor_tensor(t[:], lx[:], Dx[ky][:, :, q:q + W],
                                        ALU.mult)
                nc.vector.tensor_tensor(t2[:], lx[:], Dxy[ky][:, :, q:q + W],
                                        ALU.mult)
                if k in CCE_ADD_TAPS:
                    # t2 += Dy on the DMA engines' inline adder (SWDGE)
                    nc.gpsimd.dma_start(out=t2[:], in_=Dy[ky][:, :, q:q + W],
                                        accum_op=ALU.add)
                    pending.append((k, ky, q, t, t2, ly))
                    flush_pending(DEFER)
                else:
                    flush_pending(0)
                    nc.vector.tensor_tensor(t2[:], t2[:],
                                            Dy[ky][:, :, q:q + W], ALU.add)
                    nc.vector.tensor_tensor(t2[:], ly[:], t2[:], ALU.mult)
                    for j in range(NBLK):
                        emit_mms(k, ky, q, t, t2, j)

    nc.compile()
    return nc


def kernel(input, weight, offset):
    global _compiled
    from concourse.bass_utils import run_bass_kernel_spmd

    if _compiled is None:
        _compiled = _build()
    nc = _compiled

    input = np.asarray(input, dtype=np.float32)
    offset = np.asarray(offset, dtype=np.float32)
    w9 = np.asarray(weight, dtype=np.float32).reshape(K * K)
    wdg = np.zeros((128, K * K, 128), np.float16)
    idx = np.arange(128)
    for k in range(K * K):
        wdg[idx, k, idx] = w9[k].astype(np.float16)

    ipad = np.zeros((B, HP, WP), np.float16)
    ipad[:, 1:H + 1, 1:W + 1] = input.astype(np.float16)

    # [B, 18, 512, 512] -> [B, p, k, e, j, c] fp16, contiguous per partition
    offp = np.ascontiguousarray(
        offset.reshape(B, K * K, 2, NBLK, 128, W).transpose(0, 4, 1, 2, 3, 5)
    ).astype(np.float16)

    in_maps = [
        {"ipad": ipad[b], "offp": offp[b], "wdg": wdg} for b in range(B)
    ]
    res = run_bass_kernel_spmd(nc, in_maps, list(range(NCORES)), trace=False)
    return np.stack([res.results[b]["out"] for b in range(B)], axis=0)


# revision 26
# speedup vs baseline: 1.0583x; 1.0149x over previous
import sys

for _p in ('/opt/trn_rl_repo', '/root/.axon_site'):
    if _p not in sys.path:
        sys.path.insert(0, _p)

import numpy as np

B, H, W = 8, 512, 512
K = 3
NCORES = 8
# padded image: 1 zero row/col before, 2 zero rows/cols after (cols padded
# further so shifted views stay in range and rows stay 4B-aligned)
HP, WP = H + 3, W + 8
NBLK = 4          # row blocks of 128 partitions packed along the free dim
AW = 520          # A tile width (Ipad cols 0..519)
DW = 516          # Dx/Dy/Dxy tile width

# GpSimd tensor_tensor measured: mutual serialization with DVE TT on the
# shared SBUF port pair (combined throughput BELOW DVE alone) — keep all
# elementwise on DVE. GpSimd only generates SWDGE descriptors for the
# CCE-accumulate DMAs below.
CCE_ADD_TAPS = (2, 3, 4, 5, 6)   # taps whose t2 += Dy runs on the DMA CCE adder
DEFER = 3                        # taps to defer a CCE tap's final mul by

_compiled = None


def _build():
    import concourse.bacc as bacc
    import concourse.mybir as mybir
    from concourse.tile import TileContext

    f32, f16 = mybir.dt.float32, mybir.dt.float16
    ALU = mybir.AluOpType
    ACTF = mybir.ActivationFunctionType

    nc = bacc.Bacc("TRN2", target_bir_lowering=False, debug=False,
                   num_devices=NCORES)
    ipad = nc.dram_tensor("ipad", [HP, WP], f16, kind="ExternalInput")
    # offsets host-cast to fp16 and pre-packed to the SBUF tile layout:
    # offp[p, k, e, j, c] = offset[2k+e, 128j+p, c]  (e: 0=ly, 1=lx)
    offp = nc.dram_tensor("offp", [128, K * K, 2, NBLK, W], f16,
                          kind="ExternalInput")
    # stack of diag(w_k) matrices used as PE stationary weights
    wdg = nc.dram_tensor("wdg", [128, K * K, 128], f16, kind="ExternalInput")
    out = nc.dram_tensor("out", [H, W], f32, kind="ExternalOutput")

    with TileContext(nc) as tc:
        with (
            tc.tile_pool(name="img", bufs=1) as ip,
            tc.tile_pool(name="lylx", bufs=6) as lp,
            tc.tile_pool(name="tmp", bufs=DEFER + 2) as tp,
            tc.tile_pool(name="cst", bufs=1) as cp,
            tc.tile_pool(name="psum", bufs=1, space="PSUM") as pp,
        ):
            # image row-shifted copies (the two HWDGE rings)
            A = {}

            # per-tap (ly, lx) pair: one plain HWDGE load each, fp16 in HBM
            lylx = {}

            def load_lylx(k, eng):
                lylx[k] = lp.tile([128, 2, NBLK, W], f16, tag="l",
                                  name=f"l{k}")
                eng.dma_start(out=lylx[k][:], in_=offp[:, k])

            # j-half image chunks interleaved with offset pairs, in the
            # order the DVE stream consumes them — no engine waits long
            def load_img_half(dy, h, eng):
                if dy not in A:
                    A[dy] = ip.tile([128, NBLK, AW], f16, tag=f"A{dy}",
                                    name=f"A{dy}")
                eng.dma_start(
                    out=A[dy][:, 2 * h:2 * h + 2],
                    in_=ipad[dy + 1:dy + 513, 0:AW].rearrange(
                        "(j p) c -> p j c", p=128)[:, 2 * h:2 * h + 2])

            load_img_half(-1, 0, nc.sync)
            load_img_half(-1, 1, nc.scalar)
            load_img_half(0, 0, nc.sync)
            load_img_half(0, 1, nc.scalar)
            load_lylx(0, nc.sync)
            load_img_half(1, 0, nc.scalar)
            load_img_half(1, 1, nc.sync)
            load_lylx(1, nc.scalar)
            load_img_half(2, 0, nc.sync)
            load_img_half(2, 1, nc.scalar)
            load_lylx(2, nc.sync)
            wd = cp.tile([128, K * K, 128], f16, name="wd")
            nc.scalar.dma_start(out=wd[:], in_=wdg[:])
            for k in range(3, K * K):
                load_lylx(k, nc.sync if k % 2 == 0 else nc.scalar)

            # one PSUM tile per row block so bank j's drain never
            # serializes against bank j+1's matmuls
            psum = [pp.tile([128, W], f32, tag=f"ps{j}", name=f"ps{j}")
                    for j in range(NBLK)]

            Dx, Dy, Dxy = {}, {}, {}
            for dy in (-1, 0, 1, 2):
                Dx[dy] = ip.tile([128, NBLK, DW], f16, tag=f"D{dy}",
                                 name=f"D{dy}")
            for j in (-1, 0, 1):
                Dy[j] = ip.tile([128, NBLK, DW], f16, tag=f"Y{j}",
                                name=f"Y{j}")
                Dxy[j] = ip.tile([128, NBLK, DW], f16, tag=f"X{j}",
                                 name=f"X{j}")

            def hs(h):
                return slice(2 * h, 2 * h + 2)

            def make_dx(dy, h):
                # Dx = horizontal difference of the padded image
                nc.vector.tensor_tensor(Dx[dy][:, hs(h)],
                                        A[dy][:, hs(h), 1:1 + DW],
                                        A[dy][:, hs(h), 0:DW], ALU.subtract)

            def make_dy(j, h):
                # Dy = vertical difference of the padded image
                nc.vector.tensor_tensor(Dy[j][:, hs(h)],
                                        A[j + 1][:, hs(h), 0:DW],
                                        A[j][:, hs(h), 0:DW], ALU.subtract)

            def make_dxy(j, h):
                # Dxy = vertical difference of Dx (cross term)
                nc.vector.tensor_tensor(Dxy[j][:, hs(h)],
                                        Dx[j + 1][:, hs(h)],
                                        Dx[j][:, hs(h)], ALU.subtract)

            def iview(dy, q):
                return A[dy][:, :, q:q + W]

            outv = out.rearrange("(j p) c -> p j c", p=128)

            def emit_mms(k, ky, q, t, t2, j):
                wk = wd[:, k, :]
                last = k == K * K - 1
                nc.tensor.matmul(psum[j][:], wk, iview(ky, q)[:, j, :],
                                 start=(k == 0), stop=False)
                nc.tensor.matmul(psum[j][:], wk, t[:, j, :],
                                 start=False, stop=False)
                nc.tensor.matmul(psum[j][:], wk, t2[:, j, :],
                                 start=False, stop=last)
                if last:
                    # bank j is final: drain it while later banks finish
                    res = cp.tile([128, W], f32, tag=f"res{j}",
                                  name=f"res{j}")
                    nc.scalar.activation(res[:], psum[j][:], ACTF.Copy)
                    eng = nc.sync if j % 2 == 0 else nc.scalar
                    eng.dma_start(out=outv[:, j], in_=res[:])

            # per-row prep, j-half granularity: row ky's differences are
            # emitted just before its taps, so rows 0/1 prep hides inside
            # the tap stream while the next offset pairs arrive
            prepped = set()

            def prep_row(ky):
                for h in range(2):
                    if (ky, 'x') not in prepped:
                        make_dx(ky, h)
                    if (ky + 1, 'x') not in prepped:
                        make_dx(ky + 1, h)
                for h in range(2):
                    make_dy(ky, h)
                for h in range(2):
                    make_dxy(ky, h)
                prepped.add((ky, 'x'))
                prepped.add((ky + 1, 'x'))

            # per tap: v*w_k = w_k*I0 + w_k*m0 + w_k*u
            #   m0 = lx*Dx[ky]
            #   u  = ly*(Dy[ky] + lx*Dxy[ky])
            # CCE taps run t2 += Dy on the DMA adder; their final mul (and
            # matmuls) are deferred DEFER taps so the strict-FIFO DVE queue
            # has covering work while the DMA add completes.
            pending = []

            def flush_pending(limit):
                while pending and len(pending) > limit:
                    pk, pky, pq, pt, pt2, ply = pending.pop(0)
                    nc.vector.tensor_tensor(pt2[:], ply[:], pt2[:], ALU.mult)
                    for j in range(NBLK):
                        emit_mms(pk, pky, pq, pt, pt2, j)

            for k in range(K * K):
                ky, kx = k // K - 1, k % K - 1
                q = kx + 1
                if kx == -1:
                    prep_row(ky)
                ll = lylx[k]
                ly, lx = ll[:, 0], ll[:, 1]
                last = k == K * K - 1

                if last:
                    flush_pending(0)
                    # per-block ops so bank j's accumulation closes (and
                    # drains) while later blocks are still on the DVE
                    t = tp.tile([128, NBLK, W], f16, tag="t", name="t")
                    t2 = tp.tile([128, NBLK, W], f16, tag="t2", name="t2")
                    for j in range(NBLK):
                        nc.vector.tensor_tensor(
                            t[:, j], lx[:, j], Dx[ky][:, j, q:q + W],
                            ALU.mult)
                        nc.vector.tensor_tensor(
                            t2[:, j], lx[:, j], Dxy[ky][:, j, q:q + W],
                            ALU.mult)
                        nc.vector.tensor_tensor(
                            t2[:, j], t2[:, j], Dy[ky][:, j, q:q + W],
                            ALU.add)
                        nc.vector.tensor_tensor(t2[:, j], ly[:, j], t2[:, j],
                                                ALU.mult)
                        emit_mms(k, ky, q, t, t2, j)
                    continue

                t = tp.tile([128, NBLK, W], f16, tag="t", name="t")
                t2 = tp.tile([128, NBLK, W], f16, tag="t2", name="t2")
                nc.vector.tensor_tensor(t[:], lx[:], Dx[ky][:, :, q:q + W],
                                        ALU.mult)
                nc.vector.tensor_tensor(t2[:], lx[:], Dxy[ky][:, :, q:q + W],
                                        ALU.mult)
                if k in CCE_ADD_TAPS:
                    # t2 += Dy on the DMA engines' inline adder (SWDGE)
                    nc.gpsimd.dma_start(out=t2[:], in_=Dy[ky][:, :, q:q + W],
                                        accum_op=ALU.add)
                    pending.append((k, ky, q, t, t2, ly))
                    flush_pending(DEFER)
                else:
                    flush_pending(0)
                    nc.vector.tensor_tensor(t2[:], t2[:],
                                            Dy[ky][:, :, q:q + W], ALU.add)
                    nc.vector.tensor_tensor(t2[:], ly[:], t2[:], ALU.mult)
                    for j in range(NBLK):
                        emit_mms(k, ky, q, t, t2, j)

    nc.compile()
    return nc


def kernel(input, weight, offset):
    global _compiled
    from concourse.bass_utils import run_bass_kernel_spmd

    if _compiled is None:
        _compiled = _build()
    nc = _compiled

    input = np.asarray(input, dtype=np.float32)
    offset = np.asarray(offset, dtype=np.float32)
    w9 = np.asarray(weight, dtype=np.float32).reshape(K * K)
    wdg = np.zeros((128, K * K, 128), np.float16)
    idx = np.arange(128)
    for k in range(K * K):
        wdg[idx, k, idx] = w9[k].astype(np.float16)

    ipad = np.zeros((B, HP, WP), np.float16)
    ipad[:, 1:H + 1, 1:W + 1] = input.astype(np.float16)

    # [B, 18, 512, 512] -> [B, p, k, e, j, c] fp16, contiguous per partition
    offp = np.ascontiguousarray(
        offset.reshape(B, K * K, 2, NBLK, 128, W).transpose(0, 4, 1, 2, 3, 5)
    ).astype(np.float16)

    in_maps = [
        {"ipad": ipad[b], "offp": offp[b], "wdg": wdg} for b in range(B)
    ]
    res = run_bass_kernel_spmd(nc, in_maps, list(range(NCORES)), trace=False)
    return np.stack([res.results[b]["out"] for b in range(B)], axis=0)


# revision 29
# speedup vs baseline: 1.0653x; 1.0066x over previous
import sys

for _p in ('/opt/trn_rl_repo', '/root/.axon_site'):
    if _p not in sys.path:
        sys.path.insert(0, _p)

import numpy as np

B, H, W = 8, 512, 512
K = 3
NCORES = 8
# padded image: 1 zero row/col before, 2 zero rows/cols after (cols padded
# further so shifted views stay in range and rows stay 4B-aligned)
HP, WP = H + 3, W + 8
NBLK = 4          # row blocks of 128 partitions packed along the free dim
AW = 520          # A tile width (Ipad cols 0..519)
DW = 516          # Dx/Dy/Dxy tile width

# GpSimd tensor_tensor measured: mutual serialization with DVE TT on the
# shared SBUF port pair (combined throughput BELOW DVE alone) — keep all
# elementwise on DVE. GpSimd only generates SWDGE descriptors for the
# CCE-accumulate DMAs below.
CCE_ADD_TAPS = (2, 3, 4, 5, 6)   # taps whose t2 += Dy runs on the DMA CCE adder
DEFER = 3                        # taps to defer a CCE tap's final mul by

_compiled = None


def _build():
    import concourse.bacc as bacc
    import concourse.mybir as mybir
    from concourse.tile import TileContext

    f32, f16 = mybir.dt.float32, mybir.dt.float16
    ALU = mybir.AluOpType
    ACTF = mybir.ActivationFunctionType

    nc = bacc.Bacc("TRN2", target_bir_lowering=False, debug=False,
                   num_devices=NCORES)
    ipad = nc.dram_tensor("ipad", [HP, WP], f16, kind="ExternalInput")
    # offsets host-cast to fp16 and pre-packed to the SBUF tile layout:
    # offp[p, k, e, j, c] = offset[2k+e, 128j+p, c]  (e: 0=ly, 1=lx)
    offp = nc.dram_tensor("offp", [128, K * K, 2, NBLK, W], f16,
                          kind="ExternalInput")
    # stack of diag(w_k) matrices used as PE stationary weights
    wdg = nc.dram_tensor("wdg", [128, K * K, 128], f16, kind="ExternalInput")
    out = nc.dram_tensor("out", [H, W], f32, kind="ExternalOutput")

    with TileContext(nc) as tc:
        with (
            tc.tile_pool(name="img", bufs=1) as ip,
            tc.tile_pool(name="lylx", bufs=6) as lp,
            tc.tile_pool(name="tmp", bufs=DEFER + 2) as tp,
            tc.tile_pool(name="cst", bufs=1) as cp,
            tc.tile_pool(name="psum", bufs=1, space="PSUM") as pp,
        ):
            # image row-shifted copies (the two HWDGE rings)
            A = {}

            # per-tap (ly, lx) pair: one plain HWDGE load each, fp16 in HBM
            lylx = {}

            def load_lylx(k, eng, halves=False):
                lylx[k] = lp.tile([128, 2, NBLK, W], f16, tag="l",
                                  name=f"l{k}")
                if halves:
                    # j-halves so the split early taps start sooner
                    eng.dma_start(out=lylx[k][:, :, 0:2],
                                  in_=offp[:, k, :, 0:2])
                    eng.dma_start(out=lylx[k][:, :, 2:4],
                                  in_=offp[:, k, :, 2:4])
                else:
                    eng.dma_start(out=lylx[k][:], in_=offp[:, k])

            # j-half image chunks interleaved with offset pairs, in the
            # order the DVE stream consumes them — no engine waits long
            def load_img_half(dy, h, eng):
                if dy not in A:
                    A[dy] = ip.tile([128, NBLK, AW], f16, tag=f"A{dy}",
                                    name=f"A{dy}")
                eng.dma_start(
                    out=A[dy][:, 2 * h:2 * h + 2],
                    in_=ipad[dy + 1:dy + 513, 0:AW].rearrange(
                        "(j p) c -> p j c", p=128)[:, 2 * h:2 * h + 2])

            load_img_half(-1, 0, nc.sync)
            load_img_half(-1, 1, nc.scalar)
            load_img_half(0, 0, nc.sync)
            load_img_half(0, 1, nc.scalar)
            load_lylx(0, nc.sync, halves=True)
            load_img_half(1, 0, nc.scalar)
            load_img_half(1, 1, nc.sync)
            load_lylx(1, nc.scalar, halves=True)
            load_img_half(2, 0, nc.sync)
            load_img_half(2, 1, nc.scalar)
            load_lylx(2, nc.sync)
            wd = cp.tile([128, K * K, 128], f16, name="wd")
            nc.scalar.dma_start(out=wd[:], in_=wdg[:])
            for k in range(3, K * K):
                load_lylx(k, nc.sync if k % 2 == 0 else nc.scalar)

            # one PSUM tile per row block so bank j's drain never
            # serializes against bank j+1's matmuls
            psum = [pp.tile([128, W], f32, tag=f"ps{j}", name=f"ps{j}")
                    for j in range(NBLK)]

            Dx, Dy, Dxy = {}, {}, {}
            for dy in (-1, 0, 1, 2):
                Dx[dy] = ip.tile([128, NBLK, DW], f16, tag=f"D{dy}",
                                 name=f"D{dy}")
            for j in (-1, 0, 1):
                Dy[j] = ip.tile([128, NBLK, DW], f16, tag=f"Y{j}",
                                name=f"Y{j}")
                Dxy[j] = ip.tile([128, NBLK, DW], f16, tag=f"X{j}",
                                 name=f"X{j}")

            def hs(h):
                return slice(2 * h, 2 * h + 2)

            def make_dx(dy, h):
                # Dx = horizontal difference of the padded image
                nc.vector.tensor_tensor(Dx[dy][:, hs(h)],
                                        A[dy][:, hs(h), 1:1 + DW],
                                        A[dy][:, hs(h), 0:DW], ALU.subtract)

            def make_dy(j, h):
                # Dy = vertical difference of the padded image
                nc.vector.tensor_tensor(Dy[j][:, hs(h)],
                                        A[j + 1][:, hs(h), 0:DW],
                                        A[j][:, hs(h), 0:DW], ALU.subtract)

            def make_dxy(j, h):
                # Dxy = vertical difference of Dx (cross term)
                nc.vector.tensor_tensor(Dxy[j][:, hs(h)],
                                        Dx[j + 1][:, hs(h)],
                                        Dx[j][:, hs(h)], ALU.subtract)

            def iview(dy, q):
                return A[dy][:, :, q:q + W]

            outv = out.rearrange("(j p) c -> p j c", p=128)

            def emit_mms(k, ky, q, t, t2, j):
                wk = wd[:, k, :]
                last = k == K * K - 1
                nc.tensor.matmul(psum[j][:], wk, iview(ky, q)[:, j, :],
                                 start=(k == 0), stop=False)
                nc.tensor.matmul(psum[j][:], wk, t[:, j, :],
                                 start=False, stop=False)
                nc.tensor.matmul(psum[j][:], wk, t2[:, j, :],
                                 start=False, stop=last)
                if last:
                    # bank j is final: drain it while later banks finish
                    res = cp.tile([128, W], f32, tag=f"res{j}",
                                  name=f"res{j}")
                    nc.scalar.activation(res[:], psum[j][:], ACTF.Copy)
                    eng = nc.sync if j % 2 == 0 else nc.scalar
                    eng.dma_start(out=outv[:, j], in_=res[:])

            # per-row prep, j-half granularity: row ky's differences are
            # emitted just before its taps, so rows 0/1 prep hides inside
            # the tap stream while the next offset pairs arrive
            prepped = set()

            def prep_row(ky):
                for h in range(2):
                    if (ky, 'x') not in prepped:
                        make_dx(ky, h)
                    if (ky + 1, 'x') not in prepped:
                        make_dx(ky + 1, h)
                for h in range(2):
                    make_dy(ky, h)
                for h in range(2):
                    make_dxy(ky, h)
                prepped.add((ky, 'x'))
                prepped.add((ky + 1, 'x'))

            # per tap: v*w_k = w_k*I0 + w_k*m0 + w_k*u
            #   m0 = lx*Dx[ky]
            #   u  = ly*(Dy[ky] + lx*Dxy[ky])
            # CCE taps run t2 += Dy on the DMA adder; their final mul (and
            # matmuls) are deferred DEFER taps so the strict-FIFO DVE queue
            # has covering work while the DMA add completes.
            pending = []

            def flush_pending(limit):
                while pending and len(pending) > limit:
                    pk, pky, pq, pt, pt2, ply = pending.pop(0)
                    nc.vector.tensor_tensor(pt2[:], ply[:], pt2[:], ALU.mult)
                    for j in range(NBLK):
                        emit_mms(pk, pky, pq, pt, pt2, j)

            for k in range(K * K):
                ky, kx = k // K - 1, k % K - 1
                q = kx + 1
                if kx == -1:
                    prep_row(ky)
                ll = lylx[k]
                ly, lx = ll[:, 0], ll[:, 1]
                last = k == K * K - 1

                if last:
                    flush_pending(0)
                    # per-block ops so bank j's accumulation closes (and
                    # drains) while later blocks are still on the DVE
                    t = tp.tile([128, NBLK, W], f16, tag="t", name="t")
                    t2 = tp.tile([128, NBLK, W], f16, tag="t2", name="t2")
                    for j in range(NBLK):
                        nc.vector.tensor_tensor(
                            t[:, j], lx[:, j], Dx[ky][:, j, q:q + W],
                            ALU.mult)
                        nc.vector.tensor_tensor(
                            t2[:, j], lx[:, j], Dxy[ky][:, j, q:q + W],
                            ALU.mult)
                        nc.vector.tensor_tensor(
                            t2[:, j], t2[:, j], Dy[ky][:, j, q:q + W],
                            ALU.add)
                        nc.vector.tensor_tensor(t2[:, j], ly[:, j], t2[:, j],
                                                ALU.mult)
                        emit_mms(k, ky, q, t, t2, j)
                    continue

                t = tp.tile([128, NBLK, W], f16, tag="t", name="t")
                t2 = tp.tile([128, NBLK, W], f16, tag="t2", name="t2")
                if k <= 1:
                    # j-half pieces: start on the first half of the offset
                    # pair while its second half is still in flight
                    for h in range(2):
                        s = hs(h)
                        nc.vector.tensor_tensor(
                            t[:, s], lx[:, s], Dx[ky][:, s, q:q + W],
                            ALU.mult)
                        nc.vector.tensor_tensor(
                            t2[:, s], lx[:, s], Dxy[ky][:, s, q:q + W],
                            ALU.mult)
                        nc.vector.tensor_tensor(
                            t2[:, s], t2[:, s], Dy[ky][:, s, q:q + W],
                            ALU.add)
                        nc.vector.tensor_tensor(t2[:, s], ly[:, s],
                                                t2[:, s], ALU.mult)
                        for j in (2 * h, 2 * h + 1):
                            emit_mms(k, ky, q, t, t2, j)
                    continue

                nc.vector.tensor_tensor(t[:], lx[:], Dx[ky][:, :, q:q + W],
                                        ALU.mult)
                nc.vector.tensor_tensor(t2[:], lx[:], Dxy[ky][:, :, q:q + W],
                                        ALU.mult)
                if k in CCE_ADD_TAPS:
                    # t2 += Dy on the DMA engines' inline adder (SWDGE)
                    nc.gpsimd.dma_start(out=t2[:], in_=Dy[ky][:, :, q:q + W],
                                        accum_op=ALU.add)
                    pending.append((k, ky, q, t, t2, ly))
                    flush_pending(DEFER)
                else:
                    flush_pending(0)
                    nc.vector.tensor_tensor(t2[:], t2[:],
                                            Dy[ky][:, :, q:q + W], ALU.add)
                    nc.vector.tensor_tensor(t2[:], ly[:], t2[:], ALU.mult)
                    for j in range(NBLK):
                        emit_mms(k, ky, q, t, t2, j)

    nc.compile()
    return nc


def kernel(input, weight, offset):
    global _compiled
    from concourse.bass_utils import run_bass_kernel_spmd

    if _compiled is None:
        _compiled = _build()
    nc = _compiled

    input = np.asarray(input, dtype=np.float32)
    offset = np.asarray(offset, dtype=np.float32)
    w9 = np.asarray(weight, dtype=np.float32).reshape(K * K)
    wdg = np.zeros((128, K * K, 128), np.float16)
    idx = np.arange(128)
    for k in range(K * K):
        wdg[idx, k, idx] = w9[k].astype(np.float16)

    ipad = np.zeros((B, HP, WP), np.float16)
    ipad[:, 1:H + 1, 1:W + 1] = input.astype(np.float16)

    # [B, 18, 512, 512] -> [B, p, k, e, j, c] fp16, contiguous per partition
    offp = np.ascontiguousarray(
        offset.reshape(B, K * K, 2, NBLK, 128, W).transpose(0, 4, 1, 2, 3, 5)
    ).astype(np.float16)

    in_maps = [
        {"ipad": ipad[b], "offp": offp[b], "wdg": wdg} for b in range(B)
    ]
    res = run_bass_kernel_spmd(nc, in_maps, list(range(NCORES)), trace=False)
    return np.stack([res.results[b]["out"] for b in range(B)], axis=0)
